# revision 20
# baseline (speedup 1.0000x reference)
"""Linear attention (elu(x)+1 feature map) Bass/Tile kernel for Trainium2.

Problem: B=4, H=16, S=4096, D=64, fp32.
  Qf = elu(Q)+1; Kf = (elu(K)+1)*mask
  KV = einsum('bhsd,bhse->bhde', Kf, V); Ksum = sum_s Kf
  out = (Qf @ KV) / (Qf . Ksum)

Sharding: the 64 (b,h) pairs are data-parallel; each of the 8 cores gets 8
pairs. No collectives.

Design notes (baseline 383us -> ~130us so far):
  - sequence rows mapped s = p*32 + j (p = partition, j = chunk): every
    HBM<->SBUF DMA moves 4-8KB contiguous runs per partition (128
    descriptors instead of 2048); KV/Ksum are sums over s and out rows
    follow the same relabeling, so the remap is exact.
  - all matmul operands bf16 (PE 1 cyc/row), PSUM fp32; output DMA bf16
    (upcast to fp32 on host; rel err stays ~3e-3, gate is 2e-2)
  - two prebuilt variants: general masked kernel, and a fast path for
    mask == ones (the spec's fill) that skips the mask pipeline; chosen
    per call on the host after inspecting the actual mask values.
  - feature map: exp (ACT), relu (ACT for Q, DVE for K), min via
    tensor_scalar (4x bf16), add via tensor_tensor (2x bf16); K-side
    first so the KV matmuls start as early as possible.
  - one KV matmul per chunk covers both pairs of a group:
    lhsT=[kfA|kfB] (128), rhs=[vmA|vmB] (132: V*mask, mask col, pad)
  - bd = [[KV_A,0,ksumA,0],[0,KV_B,0,ksumB]] so phase-B output is
    [outA|outB|ZnumA|ZnumB]: per 3-chunk PSUM bank one (approx)
    reciprocal + one broadcast multiply
  - PE transposes write bf16 PSUM, copies batched 8 chunks, alternating
    DVE/ACT
"""

import numpy as np

import concourse.bass as bass
import concourse.mybir as mybir
import concourse.tile as tile
from concourse.bass_utils import run_bass_kernel_spmd
from concourse.masks import make_identity

F32 = mybir.dt.float32
BF16 = mybir.dt.bfloat16
AF = mybir.ActivationFunctionType
ALU = mybir.AluOpType

N_CORES = 8
PAIRS = 8          # (b,h) pairs per core
S = 4096
D = 64
HALVES = 2         # half-groups (DMA/elementwise granularity)
CH = 16            # chunks per half-group
CHUNKS = HALVES * CH  # 32
TPB = 8            # transpose chunks batched per PSUM bank
OB_BATCH = 3       # phase-B chunks per PSUM bank (3*130*4B <= 2KB)


def build_bass(masked: bool) -> bass.Bass:
    from concourse.bacc import Bacc
    nc = Bacc()
    Qh = nc.dram_tensor("Q", [PAIRS, S, D], F32, kind="ExternalInput")
    Kh = nc.dram_tensor("K", [PAIRS, S, D], F32, kind="ExternalInput")
    Vh = nc.dram_tensor("V", [PAIRS, S, D], F32, kind="ExternalInput")
    Mh = nc.dram_tensor("mask", [PAIRS, S], F32, kind="ExternalInput")
    Oh = nc.dram_tensor("O", [PAIRS, S, D], BF16, kind="ExternalOutput")

    # s = p*32 + j: partition-contiguous rows; [128, 2048] per pair
    Qv = [Qh[p].rearrange("(a j) d -> a (j d)", a=128) for p in range(PAIRS)]
    Kv = [Kh[p].rearrange("(a j) d -> a (j d)", a=128) for p in range(PAIRS)]
    Vv = [Vh[p].rearrange("(a j) d -> a (j d)", a=128) for p in range(PAIRS)]
    Mv = [Mh[p].rearrange("(a j) -> a j", a=128) for p in range(PAIRS)]
    Ov = [Oh[p].rearrange("(a j) d -> a (j d)", a=128) for p in range(PAIRS)]
    HB = CH * D  # elements per half per partition

    with tile.TileContext(nc) as tc:
        from contextlib import ExitStack
        with ExitStack() as ctx:
            consts = ctx.enter_context(tc.tile_pool(name="consts", bufs=1))
            qr_pool = ctx.enter_context(tc.tile_pool(name="qr", bufs=2))
            kr_pool = ctx.enter_context(tc.tile_pool(name="kr", bufs=2))
            vb_pool = ctx.enter_context(tc.tile_pool(name="vb", bufs=2))
            mb_pool = ctx.enter_context(tc.tile_pool(name="mb", bufs=2))
            e_pool = ctx.enter_context(tc.tile_pool(name="e", bufs=3))
            r_pool = ctx.enter_context(tc.tile_pool(name="r", bufs=3))
            kf_pool = ctx.enter_context(tc.tile_pool(name="kf", bufs=2))
            vm_pool = ctx.enter_context(tc.tile_pool(name="vm", bufs=2))
            qf_pool = ctx.enter_context(tc.tile_pool(name="qf", bufs=2))
            qt_pool = ctx.enter_context(tc.tile_pool(name="qt", bufs=2))
            osb_pool = ctx.enter_context(tc.tile_pool(name="osb", bufs=2))
            zr_pool = ctx.enter_context(tc.tile_pool(name="zr", bufs=3))
            kv_psum = ctx.enter_context(tc.tile_pool(name="kvps", bufs=2, space="PSUM"))
            tp_psum = ctx.enter_context(tc.tile_pool(name="tpps", bufs=2, space="PSUM"))
            ob_psum = ctx.enter_context(tc.tile_pool(name="obps", bufs=3, space="PSUM"))

            identity = consts.tile([128, 128], BF16)
            make_identity(nc, identity)
            warm = consts.tile([128, 1], F32, tag="warm", name="warm")
            nc.scalar.activation(warm, warm, AF.Exp)
            nc.scalar.activation(warm, warm, AF.Relu)
            # bd zero regions never change: two preset buffers reused by
            # alternating groups
            bds = [consts.tile([128, 130], BF16, tag=f"bd{i}", name=f"bd{i}")
                   for i in range(2)]
            for b in bds:
                nc.gpsimd.memset(b, 0.0)
            if not masked:
                # fast path: vm buffers preallocated; the Ksum column
                # (and the alignment-pad col) is constant 1.0
                vms = [consts.tile([128, CH, 2, D + 2], BF16,
                                   tag=f"vm{i}", name=f"vm{i}")
                       for i in range(2)]
                for v in vms:
                    nc.vector.memset(v[:, :, :, D:D + 2], 1.0)

            vmi = 0
            for g in range(PAIRS // 2):
                pA, pB = 2 * g, 2 * g + 1
                kv_ps = kv_psum.tile([128, 132], F32, tag="kv")
                qf = qf_pool.tile([128, CHUNKS, 2, D], BF16, tag="qf")
                qt = qt_pool.tile([128, CHUNKS, 128], BF16, tag="qt")
                if masked:
                    mb = mb_pool.tile([128, 2, CHUNKS, 1], BF16, tag="mb")
                    for u, p in ((0, pA), (1, pB)):
                        nc.gpsimd.dma_start(out=mb[:, u, :, 0], in_=Mv[p])

                # group 0 starts with a quarter-size bite so the first
                # K-tiles (256KB DMAs) finish ahead of the prefetch burst
                if g == 0:
                    bites = [(0, 8), (8, 8), (16, 16)]
                elif g == PAIRS // 2 - 1:
                    # last group: small final bites so the drain chain
                    # (last K-tile -> feature map -> KV -> phase B) is short
                    bites = [(0, CH), (CH, 8), (CH + 8, 8)]
                else:
                    bites = [(0, CH), (CH, CH)]
                for c0g, nch in bites:
                    hs = slice(c0g * D, (c0g + nch) * D)
                    kr = kr_pool.tile([128, 2, nch, D], F32, tag="kr")
                    vb = vb_pool.tile([128, 2, nch, D], BF16, tag="vb")
                    if masked:
                        vm = vm_pool.tile([128, nch, 2, D + 2], BF16, tag="vm")
                    else:
                        vm = vms[vmi % 2][:, 0:nch]
                        vmi += 1
                    qr = qr_pool.tile([128, 2, nch, D], F32, tag="qr")
                    # K for both pairs first: ek is the head of the
                    # dependency chain and needs both halves of kr
                    for u, p in ((0, pA), (1, pB)):
                        nc.sync.dma_start(
                            out=kr[:, u].rearrange("p j d -> p (j d)"),
                            in_=Kv[p][:, hs])
                    for u, p in ((0, pA), (1, pB)):
                        nc.gpsimd.dma_start(
                            out=vb[:, u].rearrange("p j d -> p (j d)"),
                            in_=Vv[p][:, hs])
                    for u, p in ((0, pA), (1, pB)):
                        nc.sync.dma_start(
                            out=qr[:, u].rearrange("p j d -> p (j d)"),
                            in_=Qv[p][:, hs])

                    # elu(x)+1 == min(exp(x),1) + relu(x); K side first so
                    # the KV matmuls can start as early as possible
                    eq = e_pool.tile([128, nch, 2, D], BF16, tag="eq")
                    ek = e_pool.tile([128, nch, 2, D], BF16, tag="ek")
                    rq = r_pool.tile([128, nch, 2, D], BF16, tag="rq")
                    rk = r_pool.tile([128, nch, 2, D], BF16, tag="rk")
                    kf = kf_pool.tile([128, nch, 2, D], BF16, tag="kf")

                    qrc = qr.rearrange("p u j d -> p j u d")
                    krc = kr.rearrange("p u j d -> p j u d")
                    nc.scalar.activation(ek, krc, AF.Exp)
                    nc.vector.tensor_scalar_max(rk, krc, 0.0)
                    nc.vector.tensor_scalar_min(ek, ek, 1.0)
                    nc.vector.tensor_add(kf, ek, rk)
                    if masked:
                        # vm = [V * mask | mask | pad]
                        mbh = mb[:, :, c0g:c0g + nch].rearrange(
                            "p u j x -> p j u x")
                        nc.vector.tensor_tensor(
                            out=vm[:, :, :, 0:D],
                            in0=vb.rearrange("p u j d -> p j u d"),
                            in1=mbh.to_broadcast([128, nch, 2, D]),
                            op=ALU.mult)
                        nc.scalar.copy(vm[:, :, :, D:D + 1], mbh)
                    else:
                        nc.vector.tensor_copy(
                            vm[:, :, :, 0:D],
                            vb.rearrange("p u j d -> p j u d"))
                    nc.scalar.activation(eq, qrc, AF.Exp)
                    nc.scalar.activation(rq, qrc, AF.Relu)
                    nc.vector.tensor_scalar_min(eq, eq, 1.0)
                    nc.vector.tensor_add(qf[:, c0g:c0g + nch], eq, rq)

                    for b in range(nch // TPB):
                        tp = tp_psum.tile([128, TPB, 128], BF16, tag="tp")
                        for c in range(TPB):
                            cl = b * TPB + c
                            cc = c0g + cl
                            # [kfA|kfB]^T @ [vmA|vmB]: diag blocks KV_A/KV_B
                            nc.tensor.matmul(
                                kv_ps,
                                lhsT=kf[:, cl].rearrange("p u d -> p (u d)"),
                                rhs=vm[:, cl].rearrange("p u e -> p (u e)"),
                                start=(cc == 0), stop=(cc == CHUNKS - 1))
                            nc.tensor.transpose(
                                tp[:, c],
                                qf[:, cc].rearrange("p u d -> p (u d)"),
                                identity)
                        qts = qt[:, cc - TPB + 1:cc + 1]
                        if (cc // TPB) % 2 == 0:
                            nc.vector.tensor_copy(qts, tp)
                        else:
                            nc.scalar.copy(qts, tp)

                # bd = [[KV_A, 0, ksumA, 0], [0, KV_B, 0, ksumB]] (128x130)
                bd = bds[g % 2]
                nc.scalar.copy(bd[0:64, 0:64], kv_ps[0:64, 0:64])
                nc.scalar.copy(bd[64:128, 64:128], kv_ps[64:128, 66:130])
                nc.scalar.copy(bd[0:64, 128:129], kv_ps[0:64, 64:65])
                nc.scalar.copy(bd[64:128, 129:130], kv_ps[64:128, 130:131])

                out_sb = osb_pool.tile([128, 2, CHUNKS, D], BF16, tag="osb")
                for j in range((CHUNKS + OB_BATCH - 1) // OB_BATCH):
                    c0 = j * OB_BATCH
                    n = min(OB_BATCH, CHUNKS - c0)
                    ob = ob_psum.tile([128, OB_BATCH, 130], F32, tag="ob")
                    for k in range(n):
                        nc.tensor.matmul(ob[:, k], lhsT=qt[:, c0 + k], rhs=bd,
                                         start=True, stop=True)
                    zr = zr_pool.tile([128, OB_BATCH, 2, 1], F32, tag="zr")
                    nc.vector.reciprocal_approx_fast(
                        zr[:, 0:n, :, 0], ob[:, 0:n, 128:130])
                    nc.vector.tensor_tensor(
                        out=out_sb[:, :, c0:c0 + n],
                        in0=ob[:, 0:n, 0:128].rearrange(
                            "p c (u e) -> p u c e", u=2),
                        in1=zr[:, 0:n].rearrange(
                            "p c u x -> p u c x").to_broadcast([128, 2, n, D]),
                        op=ALU.mult)
                # split per half so outputs start draining while the
                # second half's phase-B matmuls still run
                for u, p in ((0, pA), (1, pB)):
                    for h in range(HALVES):
                        nc.sync.dma_start(
                            out=Ov[p][:, h * HB:(h + 1) * HB],
                            in_=out_sb[:, u, bass.ts(h, CH)].rearrange(
                                "p j d -> p (j d)"))
    nc.finalize()
    return nc


_NC_CACHE = {}


def _get_nc(masked: bool):
    if masked not in _NC_CACHE:
        _NC_CACHE[masked] = build_bass(masked)
    return _NC_CACHE[masked]


def kernel(Q: np.ndarray, K: np.ndarray, V: np.ndarray, mask: np.ndarray,
           _trace: bool = False):
    B, H = 4, 16
    NP = B * H
    per = NP // N_CORES
    Qr = np.ascontiguousarray(np.asarray(Q, dtype=np.float32).reshape(NP, S, D))
    Kr = np.ascontiguousarray(np.asarray(K, dtype=np.float32).reshape(NP, S, D))
    Vr = np.ascontiguousarray(np.asarray(V, dtype=np.float32).reshape(NP, S, D))
    Mr = np.ascontiguousarray(np.asarray(mask, dtype=np.float32).reshape(NP, S))
    masked = not bool(np.all(Mr == 1.0))

    in_maps = []
    for i in range(N_CORES):
        sl = slice(i * per, (i + 1) * per)
        in_maps.append({
            "Q": np.ascontiguousarray(Qr[sl]),
            "K": np.ascontiguousarray(Kr[sl]),
            "V": np.ascontiguousarray(Vr[sl]),
            "mask": np.ascontiguousarray(Mr[sl]),
        })

    nc = _get_nc(masked)
    res = run_bass_kernel_spmd(nc, in_maps, core_ids=list(range(N_CORES)),
                               trace=_trace)
    out = np.concatenate([np.asarray(r["O"]) for r in res.results], axis=0)
    if _trace:
        kernel._last_results = res
    return out.reshape(B, H, S, D).astype(np.float32)
